# revision 37
# baseline (speedup 1.0000x reference)
"""Self-contained 2-layer GAT kernel for 8 axon-tunneled TRN2 NeuronCores.

kernel(**inputs) takes the FULL inputs and returns the FULL [50000, 64] f32
output. Nodes are sharded by destination across 8 cores. Per layer: node
features are transformed locally, AllGathered as fat rows (values + src
attention logits) in two chunks whose boundary coincides with the int16
gather-table split, then each core dma_gathers its edges' source rows (4
SWDGE queues), computes edge softmax weights (dst logits expanded through
host-precomputed fp8 one-hot masks on the PE), and scatter-adds weighted
values per destination window via mask matmuls. al_d logits stay in SBUF
between phases; masks stream from HBM instead of being built on the DVE.
"""

import numpy as np
import ml_dtypes

import concourse.bass as bass
import concourse.mybir as mybir
import concourse.tile as tile

bf16 = ml_dtypes.bfloat16
F32 = mybir.dt.float32
BF16 = mybir.dt.bfloat16
I32 = mybir.dt.int32
I16 = mybir.dt.int16
U8 = mybir.dt.uint8
FP8 = mybir.dt.float8e4
P = 128


def bcast(ap, n):
    return bass.AP(ap.tensor, ap.offset, [list(d) for d in ap.ap] + [[0, n]])


def expand_last(ap, n):
    a = [list(d) for d in ap.ap]
    assert a[-1][1] == 1, a
    a[-1] = [0, n]
    return bass.AP(ap.tensor, ap.offset, a)


class Cfg:
    def __init__(self, N=50000, IN=128, H=8, C=32, OUT=64, cores=8, gwin=2):
        assert N % cores == 0
        self.N, self.IN, self.H, self.C, self.OUT, self.cores = N, IN, H, C, OUT, cores
        self.HC = H * C            # 256
        self.S = N // cores
        self.NWIN = (self.S + P - 1) // P
        self.gwin = gwin
        # Table rows are remapped so the int16 A/B gather split coincides
        # with a 2-chunk AllGather: chunk0 = first CW windows of every core.
        self.CW = 24                          # windows in chunk 0
        self.C0 = self.CW * P                 # rows/core in chunk 0 (3072)
        self.C1 = self.S - self.C0            # rows/core in chunk 1 (3178)
        self.HALF = self.C0 * cores           # split row (24576)
        # fat row: h bf16 (HC) + als f32 (H) padded to mult of 128 int16 slots
        self.R1 = ((self.HC + 2 * H + 127) // 128) * 128      # int16 slots (384)
        self.R2 = ((self.OUT + 2 + 127) // 128) * 128         # int16 slots (128)
        self.RA1 = 64                                         # f32 slots (256B)
        self.RA2 = 64


def wrap_idxs(flat):
    """[n] (n%128==0) -> [128, n//16] int16: 16-part wrap, replicated 8x."""
    n = flat.shape[0]
    a = flat.reshape(n // 16, 16).T.astype(np.int16)
    return np.tile(a, (8, 1))


def preprocess(cfg, x, edge_index, W1, a_src1, a_dst1, W2, a_src2, a_dst2):
    N, S, H, C = cfg.N, cfg.S, cfg.H, cfg.C
    HALF = cfg.HALF
    src = np.asarray(edge_index[0], dtype=np.int64)
    dst = np.asarray(edge_index[1], dtype=np.int64)
    loops = np.arange(N, dtype=np.int64)
    src = np.concatenate([src, loops])
    dst = np.concatenate([dst, loops])

    # remap src ids to chunked table rows: core m's rows [0,C0) land in
    # chunk 0 (row m*C0 + r), rows [C0,S) in chunk 1 (HALF + m*C1 + r-C0)
    sm, sr = src // S, src % S
    src = np.where(sr < cfg.C0, sm * cfg.C0 + sr,
                   HALF + sm * cfg.C1 + (sr - cfg.C0))

    # sort by (dst, src-chunk) so each (window, half) is contiguous
    order = np.lexsort((src >= HALF, dst))
    src, dst = src[order], dst[order]

    core_bounds = np.searchsorted(dst, np.arange(cfg.cores + 1) * S)
    kA = np.zeros(cfg.NWIN, np.int64)
    kB = np.zeros(cfg.NWIN, np.int64)
    per_core = []
    for m in range(cfg.cores):
        s0, s1 = core_bounds[m], core_bounds[m + 1]
        ds, ss = dst[s0:s1] - m * S, src[s0:s1]
        wb = np.searchsorted(ds, np.arange(cfg.NWIN + 1) * P)
        na = np.zeros(cfg.NWIN, np.int64)
        nb = np.zeros(cfg.NWIN, np.int64)
        for w in range(cfg.NWIN):
            seg = ss[wb[w]:wb[w + 1]]
            a_cnt = int((seg < HALF).sum())
            na[w], nb[w] = a_cnt, len(seg) - a_cnt
        per_core.append((ds, ss, wb))
        kA = np.maximum(kA, (na + P - 1) // P)
        kB = np.maximum(kB, (nb + P - 1) // P)
    kA = np.maximum(kA, 1)
    kB = np.maximum(kB, 1)

    # column layout per group: [A_w0..A_wx | B_w0..B_wx]
    NW, G = cfg.NWIN, cfg.gwin
    groups = [(g0, min(g0 + G, NW)) for g0 in range(0, NW, G)]
    colA, colB = {}, {}
    coff = 0
    ginfo = []   # per group: (c0, KA, KB)
    for (g0, g1) in groups:
        KAg = int(kA[g0:g1].sum())
        KBg = int(kB[g0:g1].sum())
        ca = coff
        for w in range(g0, g1):
            colA[w] = ca
            ca += int(kA[w])
        cb = coff + KAg
        for w in range(g0, g1):
            colB[w] = cb
            cb += int(kB[w])
        ginfo.append((coff, KAg, KBg))
        coff += KAg + KBg
    Ktot = coff

    gsA = np.zeros((cfg.cores, P, Ktot), np.int64)   # fat idx values (pre-wrap)
    rel = np.full((cfg.cores, P, Ktot), 255.0, np.float32)
    for m in range(cfg.cores):
        ds, ss, wb = per_core[m]
        for w in range(NW):
            wsl = slice(int(wb[w]), int(wb[w + 1]))
            w_ss, w_ds = ss[wsl], ds[wsl]
            is_b = w_ss >= HALF
            for (half, cbase, kk) in ((0, colA[w], int(kA[w])), (1, colB[w], int(kB[w]))):
                sel = is_b if half else ~is_b
                e_ss, e_ds = w_ss[sel], w_ds[sel]
                cnt = len(e_ss)
                L = kk * P
                bs = np.zeros(L, np.int64)
                br = np.full(L, 255.0, np.float32)
                bs[:cnt] = e_ss - (HALF if half else 0)
                br[:cnt] = e_ds - w * P
                gsA[m, :, cbase:cbase + kk] = bs.reshape(kk, P).T
                rel[m, :, cbase:cbase + kk] = br.reshape(kk, P).T

    def build_idx_streams(m):
        ia, ib = [], []
        for gi, (g0, g1) in enumerate(groups):
            c0, KAg, KBg = ginfo[gi]
            fa = gsA[m, :, c0:c0 + KAg].T.reshape(-1)            # token i = col*128+p
            fb = gsA[m, :, c0 + KAg:c0 + KAg + KBg].T.reshape(-1)
            ia.append(wrap_idxs(fa))
            ib.append(wrap_idxs(fb))
        return (np.ascontiguousarray(np.concatenate(ia, 1)),
                np.ascontiguousarray(np.concatenate(ib, 1)))

    va1 = (W1.reshape(cfg.IN, H, C) * a_src1[None]).sum(-1).astype(np.float32)
    vb1 = (W1.reshape(cfg.IN, H, C) * a_dst1[None]).sum(-1).astype(np.float32)
    # (c,h)-major value layout: col c*H+h holds head h, channel c
    W1cm = np.ascontiguousarray(
        W1.reshape(cfg.IN, H, C).transpose(0, 2, 1).reshape(cfg.IN, H * C))
    W1p = np.concatenate([W1cm, va1, vb1], axis=1).astype(np.float32)
    W2cm = np.ascontiguousarray(
        W2.reshape(H, C, cfg.OUT).transpose(1, 0, 2).reshape(H * C, cfg.OUT))
    W2p = np.concatenate(
        [W2cm, (W2cm @ a_src2[0])[:, None], (W2cm @ a_dst2[0])[:, None]], axis=1
    ).astype(np.float32)
    nkc = cfg.HC // P
    W2pc = np.ascontiguousarray(W2p.reshape(nkc, P, cfg.OUT + 2).transpose(1, 0, 2))
    ident = np.eye(P, dtype=np.float32)
    xT = np.ascontiguousarray(np.asarray(x, np.float32).T)

    fp8 = mybir.dt.np(mybir.dt.float8e4)
    in_maps = []
    for m in range(cfg.cores):
        ia, ib = build_idx_streams(m)
        relu8 = rel[m].astype(np.uint8)
        mb = relu8[:, :, None] == np.arange(P, dtype=np.uint8)[None, None, :]
        mh = np.ascontiguousarray(
            mb.astype(np.float32).astype(fp8).reshape(P, Ktot * P))
        rfl = np.ascontiguousarray(relu8.T.reshape(1, Ktot * P))
        in_maps.append({
            "xT": np.ascontiguousarray(xT[:, m * S:(m + 1) * S]),
            "W1p": W1p, "W2p": W2pc,
            "ident": ident,
            "idxA": ia, "idxB": ib,
            "maskM": mh, "relF": rfl,
            "iotac": np.arange(P, dtype=np.float32)[:, None],
        })
    static = {
        "kA": [int(v) for v in kA], "kB": [int(v) for v in kB],
        "colA": colA, "colB": colB, "ginfo": ginfo, "groups": groups,
        "Ktot": Ktot,
        "nIA": in_maps[0]["idxA"].shape[1],
        "nIB": in_maps[0]["idxB"].shape[1],
    }
    return static, in_maps


def build_gat(nc, cfg, static, skip_ag=False, skip_gather=False,
              skip_compute=False):
    S, H, C, HC, OUT = cfg.S, cfg.H, cfg.C, cfg.HC, cfg.OUT
    R1, R2, RA1, RA2 = cfg.R1, cfg.R2, cfg.RA1, cfg.RA2
    NW = cfg.NWIN
    kA, kB = static["kA"], static["kB"]
    colA, colB = static["colA"], static["colB"]
    ginfo, groups = static["ginfo"], static["groups"]
    Ktot = static["Ktot"]
    ACC = mybir.AluOpType
    AF = mybir.ActivationFunctionType

    xT = nc.dram_tensor("xT", [cfg.IN, S], F32, kind="ExternalInput")
    W1p = nc.dram_tensor("W1p", [cfg.IN, HC + 2 * H], F32, kind="ExternalInput")
    W2p = nc.dram_tensor("W2p", [P, HC // P, OUT + 2], F32, kind="ExternalInput")
    ident = nc.dram_tensor("ident", [P, P], F32, kind="ExternalInput")
    idxA = nc.dram_tensor("idxA", [P, static["nIA"]], I16, kind="ExternalInput")
    idxB = nc.dram_tensor("idxB", [P, static["nIB"]], I16, kind="ExternalInput")
    maskM = nc.dram_tensor("maskM", [P, Ktot * P], FP8, kind="ExternalInput")
    relF = nc.dram_tensor("relF", [1, Ktot * P], U8, kind="ExternalInput")
    iotac = nc.dram_tensor("iotac", [P, 1], F32, kind="ExternalInput")
    out = nc.dram_tensor("out", [S, OUT], F32, kind="ExternalOutput")

    T1in = nc.dram_tensor("T1in", [S, R1], I16)
    T1f = nc.dram_tensor("T1f", [cfg.N, R1], I16, addr_space="Shared")
    T2in = nc.dram_tensor("T2in", [S, R2], I16)
    T2f = nc.dram_tensor("T2f", [cfg.N, R2], I16, addr_space="Shared")
    rg = [list(range(cfg.cores))]

    with tile.TileContext(nc) as tc:
        with (
            tc.tile_pool(name="const", bufs=1) as cpool,
            tc.tile_pool(name="sb", bufs=2) as sb,
            tc.tile_pool(name="gat", bufs=3) as gat,
            tc.tile_pool(name="psA", bufs=2, space="PSUM") as psA,
            tc.tile_pool(name="psW", bufs=2, space="PSUM") as psW,
            tc.tile_pool(name="psT", bufs=2, space="PSUM") as psT,
            tc.tile_pool(name="ps2", bufs=2, space="PSUM") as ps2,
        ):
            W1p_s = cpool.tile([cfg.IN, HC + 2 * H], F32)
            nc.sync.dma_start(out=W1p_s[:], in_=W1p[:])
            W2p_s = cpool.tile([P, HC // P, OUT + 2], F32)
            nc.sync.dma_start(out=W2p_s[:], in_=W2p[:])
            iotac_s = cpool.tile([P, 1], F32)
            nc.sync.dma_start(out=iotac_s[:], in_=iotac[:])
            ident_s = cpool.tile([P, P], F32)
            nc.sync.dma_start(out=ident_s[:], in_=ident[:])
            idxA_s = cpool.tile([P, static["nIA"]], I16)
            nc.sync.dma_start(out=idxA_s[:], in_=idxA[:])
            idxB_s = cpool.tile([P, static["nIB"]], I16)
            nc.sync.dma_start(out=idxB_s[:], in_=idxB[:])
            # al_d values stay on-chip: partition p of window w holds node
            # w*P + p (same mapping phase A produces)
            aldP1 = cpool.tile([P, NW, H], F32)
            nc.vector.memset(aldP1[:], 0.0)
            aldP2 = cpool.tile([P, NW, 1], F32)
            nc.vector.memset(aldP2[:], 0.0)

            # ---------------- phase A (window pairs) ----------------------
            for g0 in range(0, NW, 2):
                gw = min(2, NW - g0)
                nng = min(gw * P, S - g0 * P)
                xw = sb.tile([cfg.IN, gw * P], F32, tag="xw")
                nc.sync.dma_start(out=xw[:, :nng], in_=xT[:, g0 * P:g0 * P + nng])
                t1w = sb.tile([P, gw, R1], I16, tag="t1w")
                for wi in range(gw):
                    w = g0 + wi
                    nn = min(P, S - w * P)
                    pA = psA.tile([P, HC + 2 * H], F32, space="PSUM", tag="pA")
                    nc.tensor.matmul(out=pA[:nn, :], lhsT=xw[:, wi * P:wi * P + nn],
                                     rhs=W1p_s[:], start=True, stop=True)
                    nc.scalar.activation(out=t1w[:].bitcast(BF16)[:nn, wi, :HC],
                                         in_=pA[:nn, :HC], func=AF.Copy)
                    nc.scalar.activation(
                        out=t1w[:].bitcast(F32)[:nn, wi, HC // 2:HC // 2 + H],
                        in_=pA[:nn, HC:HC + H], func=AF.Copy)
                    nc.scalar.activation(out=aldP1[:nn, w, :],
                                         in_=pA[:nn, HC + H:HC + 2 * H], func=AF.Copy)
                nnp = min(P, nng)          # rows valid in every window slot
                t1ap = T1in[:]
                nc.sync.dma_start(
                    out=bass.AP(t1ap.tensor, t1ap.offset + g0 * P * R1,
                                [[R1, nnp], [P * R1, gw], [1, HC + 2 * H]]),
                    in_=t1w[:nnp, :, :HC + 2 * H])

            if skip_ag:
                nc.sync.dma_start(out=T1f[0:S, :], in_=T1in[:])
            else:
                nc.gpsimd.collective_compute(
                    "AllGather", ACC.bypass, replica_groups=rg,
                    ins=[T1in[0:cfg.C0, :].opt()],
                    outs=[T1f[0:cfg.HALF, :].opt()])
                nc.gpsimd.collective_compute(
                    "AllGather", ACC.bypass, replica_groups=rg,
                    ins=[T1in[cfg.C0:S, :].opt()],
                    outs=[T1f[cfg.HALF:cfg.N, :].opt()])

            def edge_layer(layer):
                """layer 1: consume T1f/aldP1, produce T2in/aldP2; layer 2:
                consume T2f/aldP2, produce out."""
                Rrow = R1 if layer == 1 else R2
                Tf = T1f if layer == 1 else T2f
                nH = H if layer == 1 else 1
                nV = HC if layer == 1 else OUT     # value width
                alsc = HC // 2 if layer == 1 else OUT // 2

                aldw_all = sb.tile([P, NW, nH], BF16, tag="aldw")
                nc.vector.tensor_copy(out=aldw_all[:],
                                      in_=(aldP1 if layer == 1 else aldP2)[:])

                iaoff = iboff = 0
                qrr = [0]     # round-robin over the 4 SWDGE queues
                for gi, (g0, g1) in enumerate(groups):
                    c0, KAg, KBg = ginfo[gi]
                    KG = KAg + KBg
                    nA, nB = KAg * P, KBg * P
                    KCH = 8
                    def chunked_gather(tile_, base_ap, idx_tile, ioff, ncols, elem):
                        for q0 in range(0, ncols, KCH):
                            qq = min(KCH, ncols - q0)
                            nq = qq * P
                            nc.gpsimd.dma_gather(
                                out_ap=tile_[:, q0:q0 + qq, :], in_ap=base_ap,
                                idxs_ap=idx_tile[:, ioff + q0 * 8:ioff + q0 * 8 + nq // 16],
                                num_idxs=nq, num_idxs_reg=nq, elem_size=elem,
                                queue_num=qrr[0] % 4)
                            qrr[0] += 1
                    hgA = gat.tile([P, KAg, Rrow], I16, tag="hgA")
                    hgB = gat.tile([P, KBg, Rrow], I16, tag="hgB")
                    if not skip_gather:
                        chunked_gather(hgA, Tf[:cfg.HALF, :], idxA_s, iaoff, KAg, Rrow)
                        chunked_gather(hgB, Tf[cfg.HALF:, :], idxB_s, iboff, KBg, Rrow)
                    elif gi == 0:
                        nc.vector.memset(hgA[:], 0)
                        nc.vector.memset(hgB[:], 0)

                    iaoff += nA // 16
                    iboff += nB // 16

                    # group-level masks, streamed from host-precomputed tables
                    Mg = gat.tile([P, KG, P], FP8, tag="mg", bufs=2)
                    nc.sync.dma_start(
                        out=Mg[:],
                        in_=maskM[:, c0 * P:(c0 + KG) * P].rearrange(
                            "p (k e) -> p k e", e=P))
                    relF_s = gat.tile([P, KG * P], U8, tag="relf", bufs=2)
                    rfap = relF[:]
                    nc.sync.dma_start(
                        out=relF_s[:],
                        in_=bass.AP(rfap.tensor, rfap.offset + c0 * P,
                                    [[0, P], [1, KG * P]]))
                    MTg = gat.tile([P, KG, P], FP8, tag="mtg", bufs=2)
                    nc.vector.tensor_scalar(
                        out=MTg[:],
                        in0=relF_s[:].rearrange("p (k e) -> p k e", e=P),
                        scalar1=iotac_s[:, 0:1], scalar2=None,
                        op0=ACC.is_equal)

                    gw = g1 - g0
                    if layer == 2:
                        pw2 = psA.tile([P, gw, nV + nH], F32, space="PSUM",
                                       tag="pA")
                        if skip_compute:
                            nc.vector.memset(pw2[:], 0.0)
                    for w in range(g0, g1):
                        nn = min(P, S - w * P)
                        segs = []
                        if kA[w]:
                            segs.append((hgA, colA[w] - c0, colA[w], kA[w]))
                        if kB[w]:
                            segs.append((hgB, colB[w] - c0 - KAg, colB[w], kB[w]))
                        if layer == 1:
                            pw = psW.tile([P, nV + nH], F32, space="PSUM", tag="pw")
                        else:
                            pw = pw2[:, w - g0, :]
                        aldw = aldw_all[:, w, :]
                        nseg = len(segs)
                        if skip_compute and layer == 1:
                            nc.vector.memset(pw[:], 0.0)
                        for si, (hg, j0, cabs, k) in enumerate(segs if not skip_compute else []):
                            hgbf = hg[:].bitcast(BF16)
                            hgf = hg[:].bitcast(F32)
                            M = Mg[:, cabs - c0:cabs - c0 + k, :]
                            MT = MTg[:, cabs - c0:cabs - c0 + k, :]
                            # per-edge al_d: one small matmul per column into a
                            # shared PSUM tile, then batched add/exp/exp/max
                            pae = psT.tile([P, k, nH], F32, space="PSUM", tag="tp")
                            for j in range(k):
                                nc.tensor.matmul(out=pae[:, j, :], lhsT=MT[:, j, :],
                                                 rhs=aldw, start=True, stop=True)
                            s_t = sb.tile([P, k, nH], F32, tag="s1")
                            nc.vector.tensor_tensor(
                                out=s_t[:], in0=hgf[:, j0:j0 + k, alsc:alsc + nH],
                                in1=pae[:], op=ACC.add)
                            e1 = sb.tile([P, k, nH], F32, tag="e1")
                            nc.scalar.activation(out=e1[:], in_=s_t[:], func=AF.Exp)
                            e2 = sb.tile([P, k, nH], F32, tag="e2")
                            nc.scalar.activation(out=e2[:], in_=s_t[:], func=AF.Exp,
                                                 scale=0.2)
                            nc.vector.tensor_tensor(
                                out=hgbf[:, j0:j0 + k, nV:nV + nH], in0=e1[:],
                                in1=e2[:], op=ACC.max)
                            if layer == 1:
                                # (c,h)-major: in1 broadcast over c sits mid-AP
                                nc.vector.tensor_tensor(
                                    out=hgbf[:, j0:j0 + k, :nV].rearrange(
                                        "p k (c h) -> p k c h", h=nH),
                                    in0=hgbf[:, j0:j0 + k, :nV].rearrange(
                                        "p k (c h) -> p k c h", h=nH),
                                    in1=bass.AP(hgbf.tensor, hgbf[:, j0:j0 + k, nV:nV + nH].offset,
                                                [list(hgbf.ap[0]),
                                                 [hgbf.ap[1][0], k], [0, C], [1, nH]]),
                                    op=ACC.mult)
                            else:
                                nc.vector.tensor_tensor(
                                    out=hgbf[:, j0:j0 + k, :nV],
                                    in0=hgbf[:, j0:j0 + k, :nV],
                                    in1=expand_last(hgbf[:, j0:j0 + k, nV:nV + 1], nV),
                                    op=ACC.mult)
                            for j in range(k):
                                nc.tensor.matmul(
                                    out=pw[:], lhsT=M[:, j, :],
                                    rhs=hgbf[:, j0 + j, :nV + nH],
                                    start=(si == 0 and j == 0),
                                    stop=(si == nseg - 1 and j == k - 1))
                        if layer == 2:
                            continue
                        dsafe = sb.tile([P, nH], F32, tag="dsafe")
                        nc.vector.tensor_scalar_max(dsafe[:], pw[:, nV:], 1e-30)
                        r_t = sb.tile([P, nH], F32, tag="r1")
                        nc.vector.reciprocal(r_t[:], dsafe[:])
                        h1 = sb.tile([P, nV], F32, tag="h1")
                        nc.vector.tensor_tensor(
                            out=h1[:].rearrange("p (c h) -> p c h", h=nH),
                            in0=pw[:, :nV].rearrange("p (c h) -> p c h", h=nH),
                            in1=bass.AP(r_t.tensor, r_t.offset,
                                        [list(r_t.ap[0]), [0, C], [1, nH]]),
                            op=ACC.mult)
                        mn = sb.tile([P, nV], F32, tag="mn")
                        nc.scalar.activation(out=mn[:], in_=h1[:], func=AF.Relu,
                                             scale=-1.0)
                        nc.scalar.activation(out=mn[:], in_=mn[:], func=AF.Exp,
                                             scale=-1.0)
                        nc.vector.tensor_scalar_add(mn[:], mn[:], -1.0)
                        h1e = sb.tile([P, nV], F32, tag="h1e")
                        nc.vector.tensor_tensor(out=h1e[:], in0=h1[:], in1=mn[:],
                                                op=ACC.max)
                        n0 = w * P
                        # fused layer-2 prep: transpose + W2 projection
                        h1T = sb.tile([P, HC // P, P], F32, tag="h1T")
                        for b in range(HC // P):
                            tp = psT.tile([P, P], F32, space="PSUM", tag="tp")
                            nc.tensor.transpose(out=tp[:], in_=h1e[:, b * P:(b + 1) * P],
                                                identity=ident_s[:])
                            nc.scalar.activation(out=h1T[:, b, :], in_=tp[:],
                                                 func=AF.Copy)
                        p2 = ps2.tile([P, OUT + 2], F32, space="PSUM", tag="p2")
                        for b in range(HC // P):
                            nc.tensor.matmul(out=p2[:], lhsT=h1T[:, b, :],
                                             rhs=W2p_s[:, b, :],
                                             start=(b == 0), stop=(b == HC // P - 1))
                        t2w = sb.tile([P, R2], I16, tag="t2w")
                        nc.vector.tensor_copy(out=t2w[:].bitcast(BF16)[:nn, :OUT],
                                              in_=p2[:nn, :OUT])
                        nc.scalar.activation(
                            out=t2w[:].bitcast(F32)[:nn, OUT // 2:OUT // 2 + 1],
                            in_=p2[:nn, OUT:OUT + 1], func=AF.Copy)
                        nc.scalar.activation(out=aldP2[:nn, w, :],
                                             in_=p2[:nn, OUT + 1:OUT + 2], func=AF.Copy)
                        nc.sync.dma_start(out=T2in[n0:n0 + nn, :OUT + 2], in_=t2w[:nn, :OUT + 2])

                    if layer == 2:
                        nng = min(gw * P, S - g0 * P)
                        nnp = min(P, nng)
                        dsafe = sb.tile([P, gw, 1], F32, tag="dsafe")
                        nc.vector.tensor_scalar_max(dsafe[:], pw2[:, :, nV:], 1e-30)
                        r_t = sb.tile([P, gw, 1], F32, tag="r1")
                        nc.vector.reciprocal(r_t[:], dsafe[:])
                        h1 = sb.tile([P, gw, nV], F32, tag="h1")
                        nc.vector.tensor_tensor(out=h1[:], in0=pw2[:, :, :nV],
                                                in1=expand_last(r_t[:], nV),
                                                op=ACC.mult)
                        mn = sb.tile([P, gw, nV], F32, tag="mn")
                        nc.scalar.activation(out=mn[:], in_=h1[:], func=AF.Relu,
                                             scale=-1.0)
                        nc.scalar.activation(out=mn[:], in_=mn[:], func=AF.Exp,
                                             scale=-1.0)
                        nc.vector.tensor_scalar_add(mn[:], mn[:], -1.0)
                        h1e = sb.tile([P, gw, nV], F32, tag="h1e")
                        nc.vector.tensor_tensor(out=h1e[:], in0=h1[:], in1=mn[:],
                                                op=ACC.max)
                        oap = out[:]
                        nc.sync.dma_start(
                            out=bass.AP(oap.tensor, oap.offset + g0 * P * OUT,
                                        [[OUT, nnp], [P * OUT, gw], [1, OUT]]),
                            in_=h1e[:nnp, :, :])

            edge_layer(1)
            if skip_ag:
                nc.sync.dma_start(out=T2f[0:S, :], in_=T2in[:])
            else:
                nc.gpsimd.collective_compute(
                    "AllGather", ACC.bypass, replica_groups=rg,
                    ins=[T2in[0:cfg.C0, :].opt()],
                    outs=[T2f[0:cfg.HALF, :].opt()])
                nc.gpsimd.collective_compute(
                    "AllGather", ACC.bypass, replica_groups=rg,
                    ins=[T2in[cfg.C0:S, :].opt()],
                    outs=[T2f[cfg.HALF:cfg.N, :].opt()])
            edge_layer(2)
    return nc


# ======================= PJRT SPMD runner =======================
import time
import numpy as np
import jax
from jax.sharding import Mesh, PartitionSpec, NamedSharding
from jax.experimental.shard_map import shard_map

import concourse.bacc as bacc
import concourse.mybir as mybir
from concourse import bass2jax
from concourse.bass2jax import (_bass_exec_p, partition_id_tensor,
                                install_neuronx_cc_hook, fast_dispatch_compile)


class CompiledSpmd:
    def __init__(self, nc, n_cores=8):
        install_neuronx_cc_hook()
        self.nc = nc
        self.n_cores = n_cores
        partition_name = nc.partition_id_tensor.name if nc.partition_id_tensor else None
        in_names, out_names, out_avals = [], [], []
        for alloc in nc.m.functions[0].allocations:
            if not isinstance(alloc, mybir.MemoryLocationSet):
                continue
            name = alloc.memorylocations[0].name
            if alloc.kind == "ExternalInput":
                if name != partition_name:
                    in_names.append(name)
            elif alloc.kind == "ExternalOutput":
                shape = tuple(alloc.tensor_shape)
                dtype = mybir.dt.np(alloc.dtype)
                out_avals.append(jax.core.ShapedArray(shape, dtype))
                out_names.append(name)
        self.in_names = list(in_names)
        self.out_names = list(out_names)
        self.out_avals = out_avals
        n_params = len(in_names)
        n_outs = len(out_avals)
        # Outputs are fully written by the kernel, so no zero-init buffers are
        # passed; the NEFF binds outputs to the custom-call result buffers.
        all_in_names = list(in_names)
        if partition_name is not None:
            all_in_names.append(partition_name)

        def _body(*args):
            operands = list(args)
            if partition_name is not None:
                operands.append(partition_id_tensor())
            outs = _bass_exec_p.bind(
                *operands,
                out_avals=tuple(out_avals),
                in_names=tuple(all_in_names),
                out_names=tuple(out_names),
                lowering_input_output_aliases=(),
                sim_require_finite=True,
                sim_require_nnan=True,
                nc=nc,
            )
            return tuple(outs)

        devices = jax.devices()[:n_cores]
        self.mesh = Mesh(np.asarray(devices), ("core",))
        in_specs = (PartitionSpec("core"),) * n_params
        out_specs = (PartitionSpec("core"),) * n_outs
        self.fn = jax.jit(
            shard_map(_body, mesh=self.mesh, in_specs=in_specs,
                      out_specs=out_specs, check_rep=False),
        )
        self.sharding = NamedSharding(self.mesh, PartitionSpec("core"))
        self._dev_inputs = None
        self._compiled = None

    def upload(self, in_maps):
        """in_maps: list of dicts (one per core). Concats along axis0, device_put."""
        assert len(in_maps) == self.n_cores
        concat = [
            np.concatenate([np.asarray(in_maps[c][n]) for c in range(self.n_cores)], axis=0)
            for n in self.in_names
        ]
        self._dev_inputs = [jax.device_put(a, self.sharding) for a in concat]
        jax.block_until_ready(self._dev_inputs)
        if self._compiled is None:
            self._compiled = fast_dispatch_compile(
                lambda: self.fn.lower(*self._dev_inputs).compile())

    def run_async(self):
        """Submit one execution without blocking; returns device arrays."""
        return self._compiled(*self._dev_inputs)

    def run(self):
        out = self.run_async()
        jax.block_until_ready(out)
        return out

    def results(self, out):
        res = []
        for c in range(self.n_cores):
            d = {}
            for i, name in enumerate(self.out_names):
                d[name] = np.asarray(out[i]).reshape(
                    self.n_cores, *self.out_avals[i].shape)[c]
            res.append(d)
        return res

    def time_exec(self, iters=5):
        ts = []
        for _ in range(iters):
            t0 = time.perf_counter()
            out = self.run()
            ts.append(time.perf_counter() - t0)
            del out
        return ts

    def time_exec_pipelined(self, iters=64):
        """Marginal per-execution time via a two-point slope: time K1 and K2
        back-to-back submissions (device queues them, single block each), and
        divide the difference by K2-K1. The fixed dispatch round-trip latency
        cancels; what remains is the steady-state cost of one more execution
        (device execution time plus per-exec RPC service cost) — an upper
        bound on HW exec time."""
        K1, K2 = iters // 4, iters
        self.run()  # warm
        t0 = time.perf_counter()
        outs = [self.run_async() for _ in range(K1)]
        jax.block_until_ready(outs[-1])
        t1 = time.perf_counter()
        outs = [self.run_async() for _ in range(K2)]
        jax.block_until_ready(outs[-1])
        t2 = time.perf_counter()
        del outs
        return ((t2 - t1) - (t1 - t0)) / (K2 - K1)


def build_compiled(build_fn, n_cores=8, **bacc_kwargs):
    t0 = time.time()
    bacc_kwargs.setdefault("num_swdge_queues", 4)
    nc = bacc.Bacc("TRN2", target_bir_lowering=False, debug=False,
                   num_devices=n_cores, **bacc_kwargs)
    build_fn(nc)
    nc.compile()
    t1 = time.time()
    print(f"[build] bass-compile {t1-t0:.1f}s", flush=True)
    return nc


# ======================= public entry point =====================
_CACHE = {}


def _get_compiled(cfg, static):
    key = (static["Ktot"], tuple(static["kA"]), tuple(static["kB"]))
    if key not in _CACHE:
        nc = build_compiled(lambda nc: build_gat(nc, cfg, static), n_cores=cfg.cores)
        _CACHE[key] = CompiledSpmd(nc, n_cores=cfg.cores)
    return _CACHE[key]


def kernel(x, edge_index, W1, a_src1, a_dst1, b1, W2, a_src2, a_dst2, b2):
    x = np.asarray(x, np.float32)
    cfg = Cfg(N=x.shape[0])
    static, in_maps = preprocess(cfg, x, np.asarray(edge_index), np.asarray(W1),
                                 np.asarray(a_src1), np.asarray(a_dst1),
                                 np.asarray(W2), np.asarray(a_src2),
                                 np.asarray(a_dst2))
    c = _get_compiled(cfg, static)
    c.upload(in_maps)
    res = c.results(c.run())
    out = np.concatenate([res[m]["out"].view(np.float32).reshape(cfg.S, cfg.OUT)
                          for m in range(cfg.cores)], axis=0)
    b2v = np.asarray(b2, np.float32)
    if b2v.any():
        out = out + b2v[None, :]
    return out

